# revision 32
# baseline (speedup 1.0000x reference)
"""Chamfer-with-normals (6D NN search) Trainium2 kernel.

Strategy (8 NeuronCores, SPMD, no collectives):
  - 8 jobs = (batch b in 0..3) x (direction in {1,2}); core = 2*b + dir.
  - Each job is a full [8192 query x 8192 db] brute-force 6D NN search.
  - Pass A: PE matmul computes q[i,j] = -dist2 = 2*x.y - |x|^2 - |y|^2 via
    K=8 augmented vectors; DVE tensor_reduce(max) -> rowmax_i.
  - Interlude: PE transpose + ScalarE(-1) puts -rowmax back as row 8 of the
    query-side K-stationary matrix (roundtrip through DRAM).
  - Pass B: PE recomputes in transposed orientation with K=9:
    z[j,i] = q[i,j] - rowmax_i <= 0, exactly 0 at the argmin (bitwise: same
    products in the same K order). ScalarE Relu(1e18*z + 1) gives an exact
    {0,1} one-hot mask; PE matmuls mask^T @ payload accumulate the selected
    db payload rows (xyz, normal, count-canary channel) in one PSUM bank.
  - Host: final per-row distances, normalization, sign-invariant normal
    metric, means. Rows whose count channel != 1 (f32 distance ties) are
    recomputed exactly on the host.

HW quirk handled: a PE LdWeights can carry at most ONE semaphore wait, so
matmuls that would need two waits are preceded by tiny 1x1 "touch" matmuls
that absorb one of the pending semaphore conditions.
"""

import sys

import numpy as np

for _p in ("/opt/trn_rl_repo", "/opt/pypackages"):
    if _p not in sys.path:
        sys.path.insert(0, _p)

B = 4
N = 8192  # queries per job
M = 8192  # database per job
P = 128
CH = 7  # payload channels: xyz(3), normal(3), count(1)
BIG = 1.0e18
EPS = 1e-12

_PROG_CACHE = {}


def _build_program(n, m):
    import concourse.bass as bass
    import concourse.tile as tile
    from concourse import mybir
    from concourse.masks import make_identity
    from concourse.tile_rust import add_dep_helper

    f32 = mybir.dt.float32
    nb = n // P  # query row blocks
    mb = m // P  # db row blocks
    n_chunks = n // 512
    m_chunks = m // 512

    nc = bass.Bass()
    ab_d = nc.dram_tensor("ab", [9, n + m], f32, kind="ExternalInput")
    pay_d = nc.dram_tensor("pay", [P, mb * CH], f32, kind="ExternalInput")
    out_d = nc.dram_tensor("tpay", [P, nb * CH], f32, kind="ExternalOutput")
    rmx_d = nc.dram_tensor("rmx", [n], f32)

    with tile.TileContext(nc) as tc:
        with (
            tc.tile_pool(name="singles", bufs=1) as singles,
            tc.tile_pool(name="touchps", bufs=1, space="PSUM") as touchps,
        ):
            ab_sb = singles.tile([9, n + m], f32)
            pay_sb = singles.tile([P, mb * CH], f32)
            ident = singles.tile([P, P], f32)
            rowmax = singles.tile([P, nb], f32)
            acc_sb = singles.tile([P, max(nb * CH, P)], f32)
            # rmx_sb aliases the (yet-unwritten) acc_sb tile to avoid a fresh
            # SBUF region whose zone tracking would pull in unrelated DMA sems
            rmx_sb = acc_sb[0:nb, 0:P]
            touch = touchps.tile([1, 1], f32, space="PSUM")

            def pe_touch(ap):
                return nc.tensor.matmul(
                    out=touch[0:1, 0:1],
                    lhsT=ap,
                    rhs=ap,
                    start=True,
                    stop=True,
                )

            make_identity(nc, ident[:])
            nc.sync.dma_start(out=ab_sb[:], in_=ab_d[:])
            nc.sync.dma_start(out=pay_sb[:], in_=pay_d[:])

            a_sb = ab_sb[:, 0:n]
            b_sb = ab_sb[:, n : n + m]

            # absorb the pay-DMA and identity-memset sems on PE early
            pe_touch(pay_sb[0:1, 0:1])
            pe_touch(ident[0:1, 0:1])

            # ---------------- Pass A: row maxima of q ----------------
            with (
                tc.tile_pool(name="qps", bufs=3, space="PSUM") as qps,
                tc.tile_pool(name="rm", bufs=4) as rmpool,
            ):
                for ib in range(nb):
                    n_rounds = m_chunks // 2
                    rm = rmpool.tile([P, n_rounds], f32)
                    for rnd in range(n_rounds):
                        q = qps.tile([P, 1024], f32, space="PSUM")
                        for u in range(2):
                            c = rnd * 2 + u
                            nc.tensor.matmul(
                                out=q[:, u * 512 : (u + 1) * 512],
                                lhsT=a_sb[0:8, ib * P : (ib + 1) * P],
                                rhs=b_sb[0:8, c * 512 : (c + 1) * 512],
                                start=True,
                                stop=True,
                            )
                        nc.vector.tensor_reduce(
                            out=rm[:, rnd : rnd + 1],
                            in_=q[:, 0:1024],
                            axis=mybir.AxisListType.X,
                            op=mybir.AluOpType.max,
                        )
                    nc.vector.tensor_reduce(
                        out=rowmax[:, ib : ib + 1],
                        in_=rm[:, 0:n_rounds],
                        axis=mybir.AxisListType.X,
                        op=mybir.AluOpType.max,
                    )

            # transpose rowmax [P, nb] -> [nb, P], negate, roundtrip to
            # row 8 of a_sb in natural i order.
            with tc.tile_pool(name="rmxps", bufs=1, space="PSUM") as rmxps:
                rmx_ps = rmxps.tile([nb, P], f32, space="PSUM")
                nc.tensor.transpose(
                    out=rmx_ps[:], in_=rowmax[:, 0:nb], identity=ident[:]
                )
                nc.scalar.activation(
                    out=rmx_sb[:],
                    in_=rmx_ps[:],
                    func=mybir.ActivationFunctionType.Copy,
                    scale=-1.0,
                )
                nc.sync.dma_start(
                    out=rmx_d[:].rearrange("(a b) -> a b", a=nb), in_=rmx_sb[:]
                )
                nc.sync.dma_start(out=ab_sb[8:9, 0:n], in_=rmx_d[None, :])

            # absorb the row-8 DMA sem on PE (base partition must be 0, so
            # read a K=9 column that overlaps row 8)
            row8_touch = pe_touch(ab_sb[0:9, 0:1])

            # ---------------- Pass B: mask + payload ----------------
            with (
                tc.tile_pool(name="zps", bufs=2, space="PSUM") as zps,
                tc.tile_pool(name="accps", bufs=1, space="PSUM") as accps,
                tc.tile_pool(name="mask", bufs=2) as maskpool,
            ):
                acc = accps.tile([P, nb * CH], f32, space="PSUM")
                # absorb the acc-bank WAR handover on PE before the real
                # accumulation group opens
                nc.tensor.matmul(
                    out=acc[0:1, 0:1],
                    lhsT=ab_sb[0:1, 0:1],
                    rhs=ab_sb[0:1, 0:1],
                    start=True,
                    stop=True,
                )
                for jb in range(mb):
                    mask = maskpool.tile([P, n], f32)
                    for rnd in range(n_chunks // 2):
                        z = zps.tile([P, 1024], f32, space="PSUM")
                        for u in range(2):
                            c = rnd * 2 + u
                            zmm = nc.tensor.matmul(
                                out=z[:, u * 512 : (u + 1) * 512],
                                lhsT=b_sb[0:9, jb * P : (jb + 1) * P],
                                rhs=a_sb[0:9, c * 512 : (c + 1) * 512],
                                start=True,
                                stop=True,
                            )
                            if jb == 0 and rnd == 0 and u == 0:
                                add_dep_helper(
                                    zmm.ins,
                                    row8_touch.ins,
                                    reason="order row8 sem absorber first",
                                )
                        nc.scalar.activation(
                            out=mask[:, rnd * 1024 : (rnd + 1) * 1024],
                            in_=z[:, 0:1024],
                            func=mybir.ActivationFunctionType.Relu,
                            scale=BIG,
                            bias=1.0,
                        )
                    for s in range(nb):
                        nc.tensor.matmul(
                            out=acc[:, s * CH : (s + 1) * CH],
                            lhsT=mask[:, s * P : (s + 1) * P],
                            rhs=pay_sb[:, jb * CH : (jb + 1) * CH],
                            start=(jb == 0),
                            stop=(jb == mb - 1),
                        )

                nc.vector.tensor_copy(acc_sb[:, 0 : nb * CH], acc[:])
                nc.sync.dma_start(out=out_d[:], in_=acc_sb[:, 0 : nb * CH])

    _strip_redundant_pe_waits(nc)
    return nc


def _strip_redundant_pe_waits(nc):
    """Drop transitively-redundant semaphore waits from PE instructions.

    A PE LdWeights can carry only ONE sync wait, but Tile's sem assignment
    is not transitively minimal: a matmul often gets both a PE self-wait
    (PSUM WAW) and a DVE/ACT wait (WAR) where the latter already implies the
    former (the consumer that frees the PSUM slot itself waited on the PE
    writes). Soundness: sem >= v means the instructions contributing the
    first v increments have *completed*, hence their own waits were
    satisfied, recursively.
    """
    f = nc.m.functions[0]
    insts = [ins for bb in f.blocks for ins in bb.instructions]
    k_of = {id(ins): k for k, ins in enumerate(insts)}

    sem_incs = {}  # sem id -> list of (cum_value, inst_idx)
    for k, ins in enumerate(insts):
        si = ins.sync_info
        if si is None:
            continue
        for up in si.on_update:
            if up.sync_type != "semaphore" or up.update_mode not in (
                "sem-inc",
                "sem-add-imm",
            ):
                continue
            lst = sem_incs.setdefault(up.id, [])
            prev = lst[-1][0] if lst else 0
            lst.append((prev + up.update_value, k))

    closure_memo = {}
    prefix_memo = {}  # sem id -> (built_upto_index, list of merged dicts)

    def merge(dst, src):
        for s, v in src.items():
            if dst.get(s, -1) < v:
                dst[s] = v

    def closure(k):
        # ticks guaranteed completed once instruction k has completed
        got = closure_memo.get(k)
        if got is not None:
            return got
        closure_memo[k] = {}  # cycle guard
        out = {}
        si = insts[k].sync_info
        if si is not None:
            for w in si.on_wait:
                if (
                    w.sync_type == "semaphore"
                    and w.wait_mode == "sem-ge-imm"
                    and w.wait_reg is None
                ):
                    merge(out, wait_implies(w.id, w.wait_value))
        closure_memo[k] = out
        return out

    def wait_implies(semid, v):
        out = {semid: v}
        lst = sem_incs.get(semid, [])
        # incremental prefix closures per sem (shared list updated in place
        # so reentrant calls see consistent partial data)
        if semid not in prefix_memo:
            prefix_memo[semid] = []
        prefs = prefix_memo[semid]
        while True:
            idx = len(prefs)
            if idx >= len(lst) or lst[idx][0] > v:
                break
            cum, j = lst[idx]
            cj = closure(j)  # may reenter and extend prefs (only below cum)
            if len(prefs) != idx:
                continue
            base = dict(prefs[-1]) if prefs else {}
            merge(base, cj)
            base[semid] = cum
            prefs.append(base)
        # largest prefix with cum <= v
        lo, hi = 0, len(lst)
        while lo < hi:
            mid = (lo + hi) // 2
            if lst[mid][0] <= v:
                lo = mid + 1
            else:
                hi = mid
        if lo > 0:
            merge(out, prefs[lo - 1])
        return out

    for attempt in range(3):
        closure_memo.clear()
        prefix_memo.clear()
        bad = _strip_pass(
            insts, sem_incs, merge, wait_implies, push_extras=(attempt == 2)
        )
        if not bad:
            return
    raise RuntimeError(
        f"instructions still have >1 sync wait after transitive "
        f"reduction: {bad[:5]} ({len(bad)} total)"
    )


def _strip_pass(insts, sem_incs, merge, wait_implies, push_extras):
    bad = []
    for k, ins in enumerate(insts):
        # sequencer pseudo-instructions support several waits (cap ~8)
        limit = 8 if type(ins).__name__ in ("InstDrain", "InstNop") else 1
        si = ins.sync_info
        if si is None or len(si.on_wait) <= limit:
            continue
        waits = list(si.on_wait)
        changed = True
        while len(waits) > 1 and changed:
            changed = False
            for wi, w in enumerate(waits):
                if not (
                    w.sync_type == "semaphore"
                    and w.wait_mode == "sem-ge-imm"
                    and w.wait_reg is None
                ):
                    continue
                implied = {}
                for wj, w2 in enumerate(waits):
                    if wj == wi:
                        continue
                    if (
                        w2.sync_type == "semaphore"
                        and w2.wait_mode == "sem-ge-imm"
                        and w2.wait_reg is None
                    ):
                        merge(implied, wait_implies(w2.id, w2.wait_value))
                if implied.get(w.id, -1) >= w.wait_value:
                    waits.pop(wi)
                    changed = True
                    break
        if len(waits) > limit and push_extras:
            # Fallback: push extra waits onto earlier same-engine
            # instructions. Safe when every increment satisfying the wait
            # sits earlier in the (topologically ordered) schedule than the
            # target instruction, so the moved wait cannot deadlock.
            def last_incrementer_pos(w):
                lst = sem_incs.get(w.id, [])
                pos = -1
                for cum, j in lst:
                    if cum > w.wait_value:
                        break
                    pos = max(pos, j)
                return pos

            waits.sort(key=last_incrementer_pos)
            keep = waits[-limit:]
            extras = waits[:-limit]
            eng = ins.engine.name
            kprev = k - 1
            while extras and kprev >= 0:
                cand = insts[kprev]
                csi = cand.sync_info
                if (
                    cand.engine.name == eng
                    and csi is not None
                    and len(csi.on_wait) == 0
                ):
                    w = extras[-1]
                    if last_incrementer_pos(w) < kprev:
                        extras.pop()
                        csi.on_wait = [w]
                        cand.sync_info = csi
                kprev -= 1
            waits = extras + keep
        if len(waits) > limit:
            bad.append((ins.name, [(w.ant_name, w.wait_value) for w in waits]))
        if len(waits) != len(si.on_wait):
            si.on_wait = waits
            ins.sync_info = si
    return bad


def _get_program(n, m):
    key = (n, m)
    if key not in _PROG_CACHE:
        _PROG_CACHE[key] = _build_program(n, m)
    return _PROG_CACHE[key]


def _l2norm(x):
    nrm = np.sqrt((x * x).sum(axis=-1, keepdims=True))
    return x / np.maximum(nrm, EPS)


def _host_inputs(q6, qsq, db6, dbsq, pay_xyz, pay_n, n, m):
    ab = np.empty((9, n + m), np.float32)
    ab[0:6, 0:n] = q6.T
    ab[6, 0:n] = qsq
    ab[7, 0:n] = 1.0
    ab[8, 0:n] = 0.0
    ab[0:6, n:] = 2.0 * db6.T
    ab[6, n:] = -1.0
    ab[7, n:] = -dbsq
    ab[8, n:] = 1.0
    pay = np.concatenate(
        [pay_xyz, pay_n, np.ones((m, 1), np.float32)], axis=1
    ).astype(np.float32)
    payb = np.ascontiguousarray(
        pay.reshape(m // P, P, CH).transpose(1, 0, 2).reshape(P, (m // P) * CH)
    )
    return {"ab": np.ascontiguousarray(ab), "pay": payb}


_LAST_RUN_INFO = {}
_RUNNER_CACHE = {}


def _get_runner(n, m, n_cores):
    """Build (once) a persistent jitted SPMD executor for the program.

    Mirrors concourse.bass2jax.run_bass_via_pjrt's multi-core path but
    caches the jitted callable so repeat kernel() calls skip re-lowering.
    """
    key = (n, m, n_cores)
    if key in _RUNNER_CACHE:
        return _RUNNER_CACHE[key]

    import jax
    from jax.experimental.shard_map import shard_map
    from jax.sharding import Mesh, PartitionSpec

    from concourse import bass2jax, mybir

    nc = _get_program(n, m)
    bass2jax.install_neuronx_cc_hook()

    partition_name = (
        nc.partition_id_tensor.name if nc.partition_id_tensor else None
    )
    in_names, out_names, out_avals, zero_outs = [], [], [], []
    for alloc in nc.m.functions[0].allocations:
        if not isinstance(alloc, mybir.MemoryLocationSet):
            continue
        name = alloc.memorylocations[0].name
        if alloc.kind == "ExternalInput":
            if name != partition_name:
                in_names.append(name)
        elif alloc.kind == "ExternalOutput":
            out_names.append(name)
            shape = tuple(alloc.tensor_shape)
            dtype = mybir.dt.np(alloc.dtype)
            out_avals.append(jax.core.ShapedArray(shape, dtype))
            zero_outs.append(np.zeros(shape, dtype))
    n_params = len(in_names)
    n_outs = len(out_avals)
    in_names_all = list(in_names) + list(out_names)
    if partition_name is not None:
        in_names_all.append(partition_name)

    def _body(*args):
        operands = list(args)
        if partition_name is not None:
            operands.append(bass2jax.partition_id_tensor())
        outs = bass2jax._bass_exec_p.bind(
            *operands,
            out_avals=tuple(out_avals),
            in_names=tuple(in_names_all),
            out_names=tuple(out_names),
            lowering_input_output_aliases=(),
            sim_require_finite=True,
            sim_require_nnan=True,
            nc=nc,
        )
        return tuple(outs)

    donate = tuple(range(n_params, n_params + n_outs))
    devices = jax.devices()[:n_cores]
    mesh = Mesh(np.asarray(devices), ("core",))
    sharded = jax.jit(
        shard_map(
            _body,
            mesh=mesh,
            in_specs=(PartitionSpec("core"),) * (n_params + n_outs),
            out_specs=(PartitionSpec("core"),) * n_outs,
            check_rep=False,
        ),
        donate_argnums=donate,
        keep_unused=True,
    )

    runner = {
        "sharded": sharded,
        "in_names": in_names,
        "out_names": out_names,
        "out_avals": out_avals,
        "zero_outs": zero_outs,
        "n_cores": n_cores,
    }
    _RUNNER_CACHE[key] = runner
    return runner


def _run_jobs(in_maps, n, m):
    import time

    n_cores = len(in_maps)
    r = _get_runner(n, m, n_cores)
    concat_in = [
        np.concatenate([m_[name] for m_ in in_maps], axis=0)
        for name in r["in_names"]
    ]
    concat_zeros = [
        np.zeros((n_cores * z.shape[0], *z.shape[1:]), z.dtype)
        for z in r["zero_outs"]
    ]
    t0 = time.time()
    out_arrs = r["sharded"](*concat_in, *concat_zeros)
    out_np = [np.asarray(a) for a in out_arrs]
    _LAST_RUN_INFO["exec_wall_ns"] = (time.time() - t0) * 1e9
    _LAST_RUN_INFO["exec_time_ns"] = None
    name_i = {name: i for i, name in enumerate(r["out_names"])}
    i = name_i["tpay"]
    av = r["out_avals"][i]
    per_core = out_np[i].reshape(n_cores, *av.shape)
    return [per_core[c] for c in range(n_cores)]


def kernel(xyz1, xyz2, normal_rebuild, normal_gt):
    xyz1 = np.asarray(xyz1, np.float32)
    xyz2 = np.asarray(xyz2, np.float32)
    normal_rebuild = np.asarray(normal_rebuild, np.float32)
    normal_gt = np.asarray(normal_gt, np.float32)
    b, n = xyz1.shape[0], xyz1.shape[1]
    m = xyz2.shape[1]

    n1 = _l2norm(normal_rebuild)
    n2 = _l2norm(normal_gt)
    p1 = np.concatenate([xyz1, n1], axis=2)
    p2 = np.concatenate([xyz2, n2], axis=2)
    sq1 = (p1 * p1).sum(axis=2)
    sq2 = (p2 * p2).sum(axis=2)

    jobs = []  # (q6, qsq, db6, dbsq, pay_xyz, pay_n, q_xyz, q_n)
    in_maps = []
    for core in range(2 * b):
        bi, d = core // 2, core % 2
        if d == 0:
            job = (p1[bi], sq1[bi], p2[bi], sq2[bi], xyz2[bi], n2[bi],
                   xyz1[bi], n1[bi])
        else:
            job = (p2[bi], sq2[bi], p1[bi], sq1[bi], xyz1[bi], n1[bi],
                   xyz2[bi], n2[bi])
        jobs.append(job)
        in_maps.append(_host_inputs(job[0], job[1], job[2], job[3],
                                    job[4], job[5], n, m))

    outs = _run_jobs(in_maps, n, m)

    xyz_sums = [0.0, 0.0]
    nrm_sums = [0.0, 0.0]
    counts = [0, 0]
    for core, raw in enumerate(outs):
        d = core % 2
        q6, qsq, db6, dbsq, pay_xyz, pay_n, q_xyz, q_n = jobs[core]
        t = np.ascontiguousarray(
            raw.reshape(P, n // P, CH).transpose(1, 0, 2).reshape(n, CH)
        )
        cnt = t[:, 6]
        t_xyz = t[:, 0:3].copy()
        t_n = t[:, 3:6].copy()
        bad = np.nonzero(cnt != 1.0)[0]
        for i in bad:
            # exact host fallback: ties or canary failure
            drow = qsq[i] + dbsq - 2.0 * (db6 @ q6[i])
            j = int(np.argmin(drow))
            t_xyz[i] = pay_xyz[j]
            t_n[i] = pay_n[j]
        xyz_d = ((q_xyz - t_xyz) ** 2).sum(axis=1)
        a = _l2norm(q_n)
        tn = _l2norm(t_n)
        nd = np.minimum(
            ((a - tn) ** 2).sum(axis=1), ((a + tn) ** 2).sum(axis=1)
        )
        xyz_sums[d] += float(xyz_d.sum())
        nrm_sums[d] += float(nd.sum())
        counts[d] += n

    xyz_out = xyz_sums[0] / counts[0] + xyz_sums[1] / counts[1]
    nrm_out = nrm_sums[0] / counts[0] + nrm_sums[1] / counts[1]
    return (np.float32(xyz_out), np.float32(nrm_out))
